# revision 1
# baseline (speedup 1.0000x reference)
"""Trainium2 Bass kernel for the self-attention block (nn_Attention).

Reference computation (per batch b, row h):
    f = x @ wf + bf; g = x @ wg + bg; h = x @ wh + bh      (1x1 convs)
    s = g @ f^T (over W); beta = softmax(s, -1); o = beta @ h
    out = gamma * o + x

Sharding: data-parallel over batch B=8, one batch element per NeuronCore.
Per core, each of the 128 rows is an independent [W=128, C=512] block.

Device dataflow per row r (all matmuls bf16, fp32 PSUM accumulation):
  - DMA xt tile [c,w] (host-pretransposed bf16, layout [r,p,k,w] so each
    partition reads one contiguous 1 KiB line) and x_row [w,c] f32.
  - fT/gT [64,w] = wf/wg^T x^T  (8 matmuls into one shared PSUM bank),
    bias added during the PSUM->SBUF copy on ScalarE (Identity+bias).
  - h [w,d] = x @ wh (4 matmuls), bh broadcast-added on VectorE.
  - sT[v,w] = f g^T transposed; A^T = exp(sT) on ScalarE (no max-subtract:
    |s| <= ~10 so fp32 exp is safe; softmax normalization deferred).
  - oU = A^T^T @ h;  Z/gamma = A^T^T @ (ones/gamma) into a spare column of
    the sT PSUM bank; reciprocal gives scale = gamma/Z directly.
  - out = oU * scale + x_row fused in one VectorE op; DMA out.
"""

import numpy as np
import ml_dtypes

import concourse.bacc as bacc
import concourse.bass as bass
import concourse.mybir as mybir
import concourse.tile as tile
from concourse.bass import ts

B, H, W, C = 8, 128, 128, 512
CK = C // 8  # 64
N_CORES = 8
KT = C // 128  # 4 contraction slices

F32 = mybir.dt.float32
BF16 = mybir.dt.bfloat16
BFDT = ml_dtypes.bfloat16
AF = mybir.ActivationFunctionType
ALU = mybir.AluOpType


def row_batch(rows: int) -> int:
    for rb in (8, 4, 2):
        if rows % rb == 0:
            return rb
    return 1


def build_nc(rows: int = H) -> bass.Bass:
    # Bacc (not raw Bass): its compile() legalizes multi-semaphore waits
    # (walrus allows at most one wait per instruction on TRN2).
    nc = bacc.Bacc(None)
    # RB rows ride in each DMA; host layouts keep every SBUF partition's
    # line contiguous (RB*2KiB f32 / RB*1KiB bf16) so transfers hit the
    # large-DMA efficiency regime.
    RB = row_batch(rows)
    nrb = rows // RB
    # x is read in bf16: it is only the residual operand, and the output is
    # bf16-rounded anyway; halving the biggest read saves ~17MB/core
    x_d = nc.dram_tensor("x", [nrb, 128, RB * C], BF16, kind="ExternalInput")
    xt_d = nc.dram_tensor("xt", [nrb, 128, RB * C], BF16, kind="ExternalInput")
    wfg_d = nc.dram_tensor("wfg", [C, 2 * CK], BF16, kind="ExternalInput")
    wh_d = nc.dram_tensor("wh", [C, C], BF16, kind="ExternalInput")
    bf_d = nc.dram_tensor("bf", [CK, 1], F32, kind="ExternalInput")
    bg_d = nc.dram_tensor("bg", [CK, 1], F32, kind="ExternalInput")
    bhb_d = nc.dram_tensor("bhb", [W, C], F32, kind="ExternalInput")
    onesg_d = nc.dram_tensor("onesg", [W, 1], BF16, kind="ExternalInput")
    # bf16 output halves write traffic (the host widens back to f32); with
    # the input/output DMA ring split this measured fastest: 349-359us vs
    # 368-438us for the fp32-output variants. Costs ~1.7e-3 relative error.
    out_d = nc.dram_tensor("out", [nrb, 128, RB * C], BF16, kind="ExternalOutput")

    with tile.TileContext(nc) as tc:
        with (
            tc.tile_pool(name="const", bufs=1) as cpool,
            tc.tile_pool(name="sb_x", bufs=4) as sb_x,
            tc.tile_pool(name="sb_xt", bufs=3) as sb_xt,
            tc.tile_pool(name="sb_fg", bufs=4) as sb_fg,
            tc.tile_pool(name="sb_h", bufs=3) as sb_h,
            tc.tile_pool(name="sb_at", bufs=3) as sb_at,
            tc.tile_pool(name="sb_out", bufs=3) as sb_out,
            tc.tile_pool(name="sb_small", bufs=6) as sb_small,
            tc.tile_pool(name="ps_m", bufs=2, space="PSUM") as ps_m,
            tc.tile_pool(name="ps_h", bufs=3, space="PSUM") as ps_h,
            tc.tile_pool(name="ps_o", bufs=3, space="PSUM") as ps_o,
        ):
            wfg_sb = cpool.tile([128, KT * 2 * CK], BF16)
            wh_sb = cpool.tile([128, KT * C], BF16)
            for k in range(KT):
                nc.sync.dma_start(
                    wfg_sb[:, ts(k, 2 * CK)], wfg_d[k * 128 : (k + 1) * 128, :]
                )
                nc.sync.dma_start(
                    wh_sb[:, ts(k, C)], wh_d[k * 128 : (k + 1) * 128, :]
                )
            bf_sb = cpool.tile([CK, 1], F32)
            nc.sync.dma_start(bf_sb[:], bf_d[:])
            bg_sb = cpool.tile([CK, 1], F32)
            nc.sync.dma_start(bg_sb[:], bg_d[:])
            bhb_sb = cpool.tile([W, C], F32)
            nc.sync.dma_start(bhb_sb[:], bhb_d[:])
            onesg_sb = cpool.tile([W, 1], BF16)
            nc.sync.dma_start(onesg_sb[:], onesg_d[:])

            for rb in range(nrb):
                # Ring split balanced by bytes: the big f32 x read rides the
                # ACT HWDGE ring (few, 1 MiB-class transfers -> little queue
                # time); xt reads + the epilogue-dependent out write ride the
                # otherwise-idle SP ring (blocking SP is free).
                x4 = sb_x.tile([128, RB * C], BF16, tag="x_row")
                nc.scalar.dma_start(x4[:], x_d[rb])
                xt4 = sb_xt.tile([128, RB * C], BF16, tag="xt16")
                nc.sync.dma_start(xt4[:], xt_d[rb])
                out4 = sb_out.tile([128, RB * C], BF16, tag="out_sb")
                for rr in range(RB):
                    xt16 = xt4[:, rr * C : (rr + 1) * C]
                    x_row = x4[:, rr * C : (rr + 1) * C]

                    # h natural [w, d]
                    h_ps = ps_h.tile([128, C], F32, tag="h")
                    for k in range(KT):
                        nc.tensor.matmul(
                            h_ps[:],
                            lhsT=xt16[:, ts(k, 128)],
                            rhs=wh_sb[:, ts(k, C)],
                            start=(k == 0),
                            stop=(k == KT - 1),
                        )
                    h16 = sb_h.tile([128, C], BF16, tag="h16")
                    nc.vector.tensor_add(h16[:], h_ps[:], bhb_sb[:])

                    # fT / gT [64, w] into one shared PSUM bank (two M=64
                    # groups; packed/batched variants measured slower or
                    # faulted the device)
                    fg_ps = ps_m.tile([CK, 256], F32, tag="m")
                    for k in range(KT):
                        nc.tensor.matmul(
                            fg_ps[:, 0:128],
                            lhsT=wfg_sb[:, ts(2 * k, CK)],
                            rhs=xt16[:, ts(k, 128)],
                            start=(k == 0),
                            stop=(k == KT - 1),
                        )
                    for k in range(KT):
                        nc.tensor.matmul(
                            fg_ps[:, 128:256],
                            lhsT=wfg_sb[:, ts(2 * k + 1, CK)],
                            rhs=xt16[:, ts(k, 128)],
                            start=(k == 0),
                            stop=(k == KT - 1),
                        )
                    f16 = sb_fg.tile([CK, 128], BF16, tag="f16")
                    nc.scalar.activation(
                        f16[:], fg_ps[:, 0:128], AF.Identity, bias=bf_sb[:]
                    )
                    g16 = sb_fg.tile([CK, 128], BF16, tag="g16")
                    nc.scalar.activation(
                        g16[:], fg_ps[:, 128:256], AF.Identity, bias=bg_sb[:]
                    )

                    # sT[v,w] in [:,0:128]; Z/gamma lands in column 128
                    st_ps = ps_m.tile([128, 129], F32, tag="m")
                    nc.tensor.matmul(
                        st_ps[:, 0:128], lhsT=f16[:], rhs=g16[:], start=True, stop=True
                    )
                    at16 = sb_at.tile([128, 128], BF16, tag="at16")
                    nc.scalar.activation(at16[:], st_ps[:, 0:128], AF.Exp)

                    # oU[w,d] = sum_v A^T[v,w] h[v,d];  Z/gamma via ones/gamma
                    o_ps = ps_o.tile([128, C], F32, tag="o")
                    nc.tensor.matmul(
                        o_ps[:], lhsT=at16[:], rhs=h16[:], start=True, stop=True
                    )
                    nc.tensor.matmul(
                        st_ps[:, 128:129],
                        lhsT=at16[:],
                        rhs=onesg_sb[:],
                        start=True,
                        stop=True,
                    )
                    scale = sb_small.tile([128, 1], F32, tag="scale")
                    nc.vector.reciprocal(scale[:], st_ps[:, 128:129])

                    nc.vector.scalar_tensor_tensor(
                        out4[:, rr * C : (rr + 1) * C],
                        o_ps[:],
                        scale[:],
                        x_row[:],
                        ALU.mult,
                        ALU.add,
                    )
                nc.sync.dma_start(out_d[rb], out4[:])
    nc.compile()
    return nc


def make_in_map(x_b: np.ndarray, wf, bf, wg, bg, wh, bh, gamma) -> dict:
    x_b = np.asarray(x_b, np.float32)
    rows = x_b.shape[0]
    # interleave wf/wg columns per k-slice: [.., 2k] -> wf, [.., 2k+1] -> wg
    wfg = np.stack([np.asarray(wf), np.asarray(wg)], axis=1)  # [C, 2, CK]
    wfg = wfg.reshape(C, 2 * CK).astype(BFDT)
    RB = row_batch(rows)
    nrb = rows // RB
    # x batched RB rows per DMA: [rb, p, rr, c] -> each partition line is
    # RB*2KiB contiguous
    x4 = np.ascontiguousarray(
        x_b.astype(BFDT)
        .reshape(nrb, RB, W, C)
        .transpose(0, 2, 1, 3)
        .reshape(nrb, 128, RB * C)
    )
    # pre-transposed x: [rb, p, rr, k, w] (p = channel-within-slice), each
    # partition line RB*KT*128*2B contiguous
    xt = np.ascontiguousarray(
        x_b.astype(BFDT)
        .reshape(nrb, RB, W, KT, 128)
        .transpose(0, 4, 1, 3, 2)
        .reshape(nrb, 128, RB * C)
    )
    gamma_f = float(np.float32(np.asarray(gamma)))
    onesg = np.full((W, 1), 1.0 / gamma_f, np.float32).astype(BFDT)
    return {
        "x": x4,
        "xt": xt,
        "wfg": wfg,
        "wh": np.asarray(wh).astype(BFDT),
        "bf": np.asarray(bf, np.float32).reshape(CK, 1),
        "bg": np.asarray(bg, np.float32).reshape(CK, 1),
        "bhb": np.ascontiguousarray(
            np.broadcast_to(np.asarray(bh, np.float32), (W, C))
        ),
        "onesg": onesg,
    }


def unbatch_out(arr: np.ndarray, rows: int) -> np.ndarray:
    """[nrb, 128, RB*C] device layout -> [rows, W, C] f32."""
    RB = row_batch(rows)
    nrb = rows // RB
    return (
        np.asarray(arr)
        .astype(np.float32)
        .reshape(nrb, 128, RB, C)
        .transpose(0, 2, 1, 3)
        .reshape(rows, W, C)
    )


_NC_CACHE: dict = {}


def run(inputs: dict, trace: bool = False, **run_kwargs):
    """Build (cached), run on 8 cores, return (out, BassKernelResults)."""
    from concourse.bass_utils import run_bass_kernel_spmd

    if "nc" not in _NC_CACHE:
        _NC_CACHE["nc"] = build_nc()
    nc = _NC_CACHE["nc"]
    x = np.asarray(inputs["x"], np.float32)
    in_maps = [
        make_in_map(
            x[b],
            inputs["wf"],
            inputs["bf"],
            inputs["wg"],
            inputs["bg"],
            inputs["wh"],
            inputs["bh"],
            inputs["gamma"],
        )
        for b in range(N_CORES)
    ]
    res = run_bass_kernel_spmd(
        nc, in_maps, list(range(N_CORES)), trace=trace, **run_kwargs
    )
    out = np.stack(
        [unbatch_out(res.results[b]["out"], H) for b in range(N_CORES)], axis=0
    )
    return out, res


def kernel(**inputs) -> np.ndarray:
    out, _ = run(inputs, trace=False)
    return out



# revision 5
# speedup vs baseline: 1.1845x; 1.1845x over previous
"""Trainium2 Bass kernel for the self-attention block (nn_Attention).

Reference computation (per batch b, row h):
    f = x @ wf + bf; g = x @ wg + bg; h = x @ wh + bh      (1x1 convs)
    s = g @ f^T (over W); beta = softmax(s, -1); o = beta @ h
    out = gamma * o + x

Sharding: data-parallel over batch B=8, one batch element per NeuronCore.
Per core, each of the 128 rows is an independent [W=128, C=512] block.

v2 dataflow (fp8 DoubleRow matmuls for the projections):
  - x is shipped twice: xt8 (fp8e4, transposed, DoubleRow-interleaved
    layout, 8.4MB) feeds the PE; x4 (bf16 natural, 16.8MB) is the
    residual. Output bf16 (16.8MB). Total DMA 42MB/core.
  - Weights are pre-scaled by 64 on the host so fp8e4 holds them out of
    the subnormal range (std 0.02 -> 1.28); the 1/64 (and 1/gamma) are
    folded into the exp's scale/bias arguments, so no extra device ops.
  - f^T/g^T for 4 rows at a time via 2+2 DoubleRow matmuls (M=64 so both
    land on partitions 0:63, as required by the s matmuls).
  - h[w,d] per row via 2 DoubleRow matmuls (contraction 2x128 each).
  - s and s^T both computed (N=128 bf16); exp(s^T)->at16 feeds the o
    matmul; exp(s_nat) is emitted only for its accum_out row-sum, which
    yields Z (the softmax denominator) for free - no N=1 Z-matmul.
  - bh is folded into the residual on the host (x4 = x + gamma*bh), and
    sign(gamma) into wh, so h needs no bias add and gamma<0 still works.
  - o matmul stays bf16 (its lhsT is per-row data, so DoubleRow would be
    LDWEIGHTS-bound and win nothing).
  - Engine balance per row: PE ~800ns, DVE ~700ns (h copy, recip,
    epilogue), ACT ~460ns (2 exps + f copy), GpSimd ~250ns (g copy).
"""

import math

import numpy as np
import ml_dtypes

import concourse.bacc as bacc
import concourse.bass as bass
import concourse.mybir as mybir
import concourse.tile as tile

B, H, W, C = 8, 128, 128, 512
CK = C // 8  # 64
N_CORES = 8

F32 = mybir.dt.float32
BF16 = mybir.dt.bfloat16
FP8 = mybir.dt.float8e4
BFDT = ml_dtypes.bfloat16
E4DT = ml_dtypes.float8_e4m3
AF = mybir.ActivationFunctionType
ALU = mybir.AluOpType
DR = mybir.MatmulPerfMode.DoubleRow

WS = 64.0  # host-side weight scale (keeps fp8 weights out of subnormals)


def row_batch(rows: int) -> int:
    for rb in (4, 2):
        if rows % rb == 0:
            return rb
    return 1


def build_nc(rows: int = H) -> bass.Bass:
    nc = bacc.Bacc(None)
    RB = row_batch(rows)
    nrb = rows // RB
    # xt8: fp8 transposed x, DoubleRow layout [p, j, i, r, w] per row-batch
    # (channel c = (2j+i)*128+p); each partition line RB*C bytes contiguous.
    xt8_d = nc.dram_tensor("xt8", [nrb, 128, RB * C], FP8, kind="ExternalInput")
    # x4: bf16 natural-layout x (+gamma*bh), residual operand.
    x4_d = nc.dram_tensor("x4", [nrb, 128, RB * C], BF16, kind="ExternalInput")
    wh8_d = nc.dram_tensor("wh8", [128, 2 * 2 * C], FP8, kind="ExternalInput")
    wf8_d = nc.dram_tensor("wf8", [128, 2 * 2 * CK], FP8, kind="ExternalInput")
    wg8_d = nc.dram_tensor("wg8", [128, 2 * 2 * CK], FP8, kind="ExternalInput")
    bf64_d = nc.dram_tensor("bf64", [CK, 1], F32, kind="ExternalInput")
    bg64_d = nc.dram_tensor("bg64", [CK, 1], F32, kind="ExternalInput")
    lng_d = nc.dram_tensor("lng", [128, 1], F32, kind="ExternalInput")
    out_d = nc.dram_tensor("out", [nrb, 128, RB * C], BF16, kind="ExternalOutput")

    with tile.TileContext(nc) as tc:
        with (
            tc.tile_pool(name="const", bufs=1) as cpool,
            tc.tile_pool(name="sb_xt", bufs=3) as sb_xt,
            tc.tile_pool(name="sb_x", bufs=3) as sb_x,
            tc.tile_pool(name="sb_fg", bufs=2) as sb_fg,
            tc.tile_pool(name="sb_h", bufs=3) as sb_h,
            tc.tile_pool(name="sb_at", bufs=3) as sb_at,
            tc.tile_pool(name="sb_an", bufs=2) as sb_an,
            tc.tile_pool(name="sb_out", bufs=2) as sb_out,
            tc.tile_pool(name="sb_small", bufs=8) as sb_small,
            tc.tile_pool(name="ps_f", bufs=1, space="PSUM") as ps_f,
            tc.tile_pool(name="ps_g", bufs=1, space="PSUM") as ps_g,
            tc.tile_pool(name="ps_h", bufs=2, space="PSUM") as ps_h,
            tc.tile_pool(name="ps_s", bufs=2, space="PSUM") as ps_s,
            tc.tile_pool(name="ps_o", bufs=2, space="PSUM") as ps_o,
        ):
            # --- static weights/biases ---
            wh8_sb = cpool.tile([128, 2, 2, C], FP8)
            nc.sync.dma_start(wh8_sb[:], wh8_d[:])
            wf8_sb = cpool.tile([128, 2, 2, CK], FP8)
            nc.sync.dma_start(wf8_sb[:], wf8_d[:])
            wg8_sb = cpool.tile([128, 2, 2, CK], FP8)
            nc.sync.dma_start(wg8_sb[:], wg8_d[:])
            bf64_sb = cpool.tile([CK, 1], F32)
            nc.sync.dma_start(bf64_sb[:], bf64_d[:])
            bg64_sb = cpool.tile([CK, 1], F32)
            nc.sync.dma_start(bg64_sb[:], bg64_d[:])
            lng_sb = cpool.tile([128, 1], F32)
            nc.sync.dma_start(lng_sb[:], lng_d[:])

            for rb in range(nrb):
                # input DMAs ride separate descriptor rings: xt8 on the
                # (otherwise idle) GpSimd ring, x4 on the ACT ring, out on SP
                xt8 = sb_xt.tile([128, 2, 2, RB, 128], FP8, tag="xt8")
                nc.gpsimd.dma_start(xt8[:], xt8_d[rb])
                x4 = sb_x.tile([128, RB * C], BF16, tag="x4")
                nc.scalar.dma_start(x4[:], x4_d[rb])
                out4 = sb_out.tile([128, RB * C], BF16, tag="out4")

                # fT/gT for the whole row-batch: DR matmuls, M=64 so both
                # land on partitions 0:63 (required by the s matmuls below)
                fA = ps_f.tile([CK, RB * 128], F32, tag="fA")
                gA = ps_g.tile([CK, RB * 128], F32, tag="gA")
                for j in range(2):
                    nc.tensor.matmul(
                        fA[:], lhsT=wf8_sb[:, j], rhs=xt8[:, j],
                        start=(j == 0), stop=(j == 1), perf_mode=DR,
                    )
                for j in range(2):
                    nc.tensor.matmul(
                        gA[:], lhsT=wg8_sb[:, j], rhs=xt8[:, j],
                        start=(j == 0), stop=(j == 1), perf_mode=DR,
                    )
                ft16 = sb_fg.tile([CK, RB, 128], BF16, tag="ft16")
                nc.scalar.activation(ft16[:], fA[:], AF.Identity, bias=bf64_sb[:])
                gt16 = sb_fg.tile([CK, RB, 128], BF16, tag="gt16")
                nc.scalar.activation(gt16[:], gA[:], AF.Identity, bias=bg64_sb[:])

                # software-pipelined across rows: h(r+1) issues between the
                # s matmuls and o(r) so the PE never waits on the exp latency
                h_ps_list = [None] * RB
                h_ps_list[0] = ps_h.tile([128, C], F32, tag="h", name="h_ps")
                for j in range(2):
                    nc.tensor.matmul(
                        h_ps_list[0][:], lhsT=xt8[:, j, :, 0, :], rhs=wh8_sb[:, j],
                        start=(j == 0), stop=(j == 1), perf_mode=DR,
                    )
                for r in range(RB):
                    h16 = sb_h.tile([128, C], BF16, tag="h16")
                    nc.vector.tensor_copy(h16[:], h_ps_list[r][:])

                    s_ps = ps_s.tile([128, 256], F32, tag="s")
                    nc.tensor.matmul(
                        s_ps[:, 0:128], lhsT=gt16[:, r], rhs=ft16[:, r],
                        start=True, stop=True,
                    )
                    nc.tensor.matmul(
                        s_ps[:, 128:256], lhsT=ft16[:, r], rhs=gt16[:, r],
                        start=True, stop=True,
                    )
                    if r + 1 < RB:
                        h_ps_list[r + 1] = ps_h.tile(
                            [128, C], F32, tag="h", name="h_ps"
                        )
                        for j in range(2):
                            nc.tensor.matmul(
                                h_ps_list[r + 1][:], lhsT=xt8[:, j, :, r + 1, :],
                                rhs=wh8_sb[:, j],
                                start=(j == 0), stop=(j == 1), perf_mode=DR,
                            )

                    at16 = sb_at.tile([128, 128], BF16, tag="at16")
                    nc.scalar.activation(
                        at16[:], s_ps[:, 128:256], AF.Exp, scale=1.0 / (WS * WS)
                    )
                    # exp(s_nat) emitted only for its accumulator: Z*64/|gamma|
                    an = sb_an.tile([128, 128], BF16, tag="an")
                    zs = sb_small.tile([128, 1], F32, tag="zs")
                    nc.scalar.activation(
                        an[:], s_ps[:, 0:128], AF.Exp,
                        scale=1.0 / (WS * WS), bias=lng_sb[:], accum_out=zs[:],
                    )
                    scale = sb_small.tile([128, 1], F32, tag="scale")
                    nc.vector.reciprocal(scale[:], zs[:])

                    o_ps = ps_o.tile([128, C], F32, tag="o")
                    nc.tensor.matmul(
                        o_ps[:], lhsT=at16[:], rhs=h16[:], start=True, stop=True
                    )
                    nc.vector.scalar_tensor_tensor(
                        out4[:, r * C : (r + 1) * C],
                        o_ps[:],
                        scale[:],
                        x4[:, r * C : (r + 1) * C],
                        ALU.mult,
                        ALU.add,
                    )
                nc.sync.dma_start(out_d[rb], out4[:])
    nc.compile()
    return nc


def make_in_map(x_b: np.ndarray, wf, bf, wg, bg, wh, bh, gamma) -> dict:
    """Host-side input staging for one core (one batch element).

    All transforms are layout/dtype-only plus constant folds:
      - weights scaled by 64 (fp8 subnormal avoidance), sign(gamma) folded
        into wh, gamma*bh folded into the residual copy of x.
    """
    x_b = np.asarray(x_b, np.float32)
    rows = x_b.shape[0]
    RB = row_batch(rows)
    nrb = rows // RB
    gamma_f = float(np.float32(np.asarray(gamma)))
    sgn = 1.0 if gamma_f >= 0 else -1.0
    ag = max(abs(gamma_f), 1e-30)

    # xt8 [nrb, p, jj, r, w]: val = x[rb*RB+r, w, jj*128+p]
    xt8 = np.ascontiguousarray(
        x_b.astype(E4DT)
        .reshape(nrb, RB, W, 4, 128)
        .transpose(0, 4, 3, 1, 2)
        .reshape(nrb, 128, RB * C)
    )
    # x4 [nrb, w, (r, c)] bf16 with gamma*bh folded in
    x_adj = x_b + gamma_f * np.asarray(bh, np.float32)
    x4 = np.ascontiguousarray(
        x_adj.astype(BFDT)
        .reshape(nrb, RB, W, C)
        .transpose(0, 2, 1, 3)
        .reshape(nrb, 128, RB * C)
    )

    def w_dr(w_mat, scale):
        # [C, M] -> [p, jj, M] -> [128, 2*2*M] fp8
        w_mat = np.asarray(w_mat, np.float32) * scale
        m = w_mat.shape[1]
        return np.ascontiguousarray(
            w_mat.astype(E4DT).reshape(4, 128, m).transpose(1, 0, 2).reshape(128, 4 * m)
        )

    return {
        "xt8": xt8,
        "x4": x4,
        "wh8": w_dr(wh, WS * sgn),
        "wf8": w_dr(wf, WS),
        "wg8": w_dr(wg, WS),
        "bf64": np.asarray(bf, np.float32).reshape(CK, 1) * WS,
        "bg64": np.asarray(bg, np.float32).reshape(CK, 1) * WS,
        "lng": np.full((128, 1), math.log(WS / ag), np.float32),
    }


def unbatch_out(arr: np.ndarray, rows: int) -> np.ndarray:
    """[nrb, 128, RB*C] device layout -> [rows, W, C] f32."""
    RB = row_batch(rows)
    nrb = rows // RB
    return (
        np.asarray(arr)
        .astype(np.float32)
        .reshape(nrb, 128, RB, C)
        .transpose(0, 2, 1, 3)
        .reshape(rows, W, C)
    )


_NC_CACHE: dict = {}


def run(inputs: dict, trace: bool = False, **run_kwargs):
    """Build (cached), run on 8 cores, return (out, BassKernelResults)."""
    from concourse.bass_utils import run_bass_kernel_spmd

    if "nc" not in _NC_CACHE:
        _NC_CACHE["nc"] = build_nc()
    nc = _NC_CACHE["nc"]
    x = np.asarray(inputs["x"], np.float32)
    in_maps = [
        make_in_map(
            x[b],
            inputs["wf"],
            inputs["bf"],
            inputs["wg"],
            inputs["bg"],
            inputs["wh"],
            inputs["bh"],
            inputs["gamma"],
        )
        for b in range(N_CORES)
    ]
    res = run_bass_kernel_spmd(
        nc, in_maps, list(range(N_CORES)), trace=trace, **run_kwargs
    )
    out = np.stack(
        [unbatch_out(res.results[b]["out"], H) for b in range(N_CORES)], axis=0
    )
    return out, res


def kernel(**inputs) -> np.ndarray:
    out, _ = run(inputs, trace=False)
    return out


# revision 7
# speedup vs baseline: 1.2528x; 1.0576x over previous
"""Trainium2 Bass kernel for the self-attention block (nn_Attention).

Reference computation (per batch b, row h):
    f = x @ wf + bf; g = x @ wg + bg; h = x @ wh + bh      (1x1 convs)
    s = g @ f^T (over W); beta = softmax(s, -1); o = beta @ h
    out = gamma * o + x

Sharding: data-parallel over batch B=8, one batch element per NeuronCore.
Per core, each of the 128 rows is an independent [W=128, C=512] block.

v3 dataflow. fp8e4 DoubleRow matmuls for the projections; all hot
PSUM->SBUF drains tuned against measured engine rates (ACT ~263+1.07/col,
DVE ~190+1.18/col, both much slower than arch spec due to the cayman
SBUF-access errata):
  - x ships twice: xt8 (fp8, transposed + DoubleRow-interleaved, 8.4MB)
    feeds the PE; x4 (bf16 natural, 16.8MB) is the residual. out bf16.
  - Weights pre-scaled by 64 on the host (fp8 subnormal avoidance); the
    1/64**2 is folded into the exp scale, the 64/|gamma| into the ones
    vector of the Z-matmul, sign(gamma) into wh, gamma*bh into x4.
  - fT/gT for 4 rows via 2+2 DoubleRow MMs (M=64 -> partitions 0:63).
  - h per row: 2 DoubleRow MMs; rows are processed in PAIRS sharing one
    [128,2x512] PSUM tile so the drain (split ACT/DVE by H_ACT) runs as
    one op per engine per pair.
  - s^T for the pair lands in one [128,256] PSUM tile -> ONE exp (ACT)
    -> at2; softmax denominator Z via two N=1 matmuls with the at2
    halves as stationary (reuses the o-matmul weights, no extra exp,
    no accum_out) -> one paired reciprocal (DVE).
  - o matmul bf16 per row; epilogue = one DVE scalar_tensor_tensor
    (o*scale + x4) per row.
"""

import math

import numpy as np
import ml_dtypes

import concourse.bacc as bacc
import concourse.bass as bass
import concourse.mybir as mybir
import concourse.tile as tile

B, H, W, C = 8, 128, 128, 512
CK = C // 8  # 64
N_CORES = 8

F32 = mybir.dt.float32
BF16 = mybir.dt.bfloat16
FP8 = mybir.dt.float8e4
BFDT = ml_dtypes.bfloat16
E4DT = ml_dtypes.float8_e4m3
AF = mybir.ActivationFunctionType
ALU = mybir.AluOpType
DR = mybir.MatmulPerfMode.DoubleRow

WS = 64.0   # host-side weight scale
H_ACT = 320  # columns of each h drain handled by ScalarE (rest on VectorE)


def row_batch(rows: int) -> int:
    for rb in (4, 2):
        if rows % rb == 0:
            return rb
    return 1


def build_nc(rows: int = H) -> bass.Bass:
    nc = bacc.Bacc(None)
    RB = row_batch(rows)
    nrb = rows // RB
    npair = RB // 2 if RB % 2 == 0 else 0
    assert npair, "rows must be a multiple of 2"
    xt8_d = nc.dram_tensor("xt8", [nrb, 128, RB * C], FP8, kind="ExternalInput")
    x4_d = nc.dram_tensor("x4", [nrb, 128, RB * C], BF16, kind="ExternalInput")
    wh8_d = nc.dram_tensor("wh8", [128, 2 * 2 * C], FP8, kind="ExternalInput")
    wf8_d = nc.dram_tensor("wf8", [128, 2 * 2 * CK], FP8, kind="ExternalInput")
    wg8_d = nc.dram_tensor("wg8", [128, 2 * 2 * CK], FP8, kind="ExternalInput")
    bf64_d = nc.dram_tensor("bf64", [CK, 1], F32, kind="ExternalInput")
    bg64_d = nc.dram_tensor("bg64", [CK, 1], F32, kind="ExternalInput")
    onesg_d = nc.dram_tensor("onesg", [W, 1], BF16, kind="ExternalInput")
    out_d = nc.dram_tensor("out", [nrb, 128, RB * C], BF16, kind="ExternalOutput")

    with tile.TileContext(nc) as tc:
        with (
            tc.tile_pool(name="const", bufs=1) as cpool,
            tc.tile_pool(name="sb_xt", bufs=3) as sb_xt,
            tc.tile_pool(name="sb_x", bufs=3) as sb_x,
            tc.tile_pool(name="sb_fg", bufs=2) as sb_fg,
            tc.tile_pool(name="sb_h", bufs=2) as sb_h,
            tc.tile_pool(name="sb_at", bufs=3) as sb_at,
            tc.tile_pool(name="sb_out", bufs=2) as sb_out,
            tc.tile_pool(name="sb_small", bufs=6) as sb_small,
            tc.tile_pool(name="ps_f", bufs=1, space="PSUM") as ps_f,
            tc.tile_pool(name="ps_g", bufs=1, space="PSUM") as ps_g,
            tc.tile_pool(name="ps_h", bufs=1, space="PSUM") as ps_h,
            tc.tile_pool(name="ps_s", bufs=2, space="PSUM") as ps_s,
            tc.tile_pool(name="ps_o", bufs=2, space="PSUM") as ps_o,
        ):
            wh8_sb = cpool.tile([128, 2, 2, C], FP8)
            nc.sync.dma_start(wh8_sb[:], wh8_d[:])
            wf8_sb = cpool.tile([128, 2, 2, CK], FP8)
            nc.sync.dma_start(wf8_sb[:], wf8_d[:])
            wg8_sb = cpool.tile([128, 2, 2, CK], FP8)
            nc.sync.dma_start(wg8_sb[:], wg8_d[:])
            bf64_sb = cpool.tile([CK, 1], F32)
            nc.sync.dma_start(bf64_sb[:], bf64_d[:])
            bg64_sb = cpool.tile([CK, 1], F32)
            nc.sync.dma_start(bg64_sb[:], bg64_d[:])
            onesg_sb = cpool.tile([W, 1], BF16)
            nc.sync.dma_start(onesg_sb[:], onesg_d[:])

            def emit_h_pair(xt8_t, p):
                """h for rows (2p, 2p+1) into one [128, 2, C] PSUM tile."""
                hp = ps_h.tile([128, 2, C], F32, tag="h", name="h_ps")
                for rr in range(2):
                    for j in range(2):
                        nc.tensor.matmul(
                            hp[:, rr], lhsT=xt8_t[:, j, :, 2 * p + rr, :],
                            rhs=wh8_sb[:, j],
                            start=(j == 0), stop=(j == 1), perf_mode=DR,
                        )
                return hp

            for rb in range(nrb):
                xt8 = sb_xt.tile([128, 2, 2, RB, 128], FP8, tag="xt8")
                nc.gpsimd.dma_start(xt8[:], xt8_d[rb])
                x4 = sb_x.tile([128, RB * C], BF16, tag="x4")
                nc.scalar.dma_start(x4[:], x4_d[rb])
                out4 = sb_out.tile([128, RB * C], BF16, tag="out4")

                fA = ps_f.tile([CK, RB * 128], F32, tag="fA")
                gA = ps_g.tile([CK, RB * 128], F32, tag="gA")
                for j in range(2):
                    nc.tensor.matmul(
                        fA[:], lhsT=wf8_sb[:, j], rhs=xt8[:, j],
                        start=(j == 0), stop=(j == 1), perf_mode=DR,
                    )
                for j in range(2):
                    nc.tensor.matmul(
                        gA[:], lhsT=wg8_sb[:, j], rhs=xt8[:, j],
                        start=(j == 0), stop=(j == 1), perf_mode=DR,
                    )
                ft16 = sb_fg.tile([CK, RB, 128], BF16, tag="ft16")
                nc.scalar.activation(ft16[:], fA[:], AF.Identity, bias=bf64_sb[:])
                gt16 = sb_fg.tile([CK, RB, 128], BF16, tag="gt16")
                nc.scalar.activation(gt16[:], gA[:], AF.Identity, bias=bg64_sb[:])

                hp = emit_h_pair(xt8, 0)
                for p in range(npair):
                    # h drain for the pair, split ACT/DVE by column range
                    h2 = sb_h.tile([128, 2, C], BF16, tag="h2")
                    nc.scalar.activation(
                        h2[:, :, 0:H_ACT], hp[:, :, 0:H_ACT], AF.Identity
                    )
                    nc.vector.tensor_copy(h2[:, :, H_ACT:C], hp[:, :, H_ACT:C])

                    s_ps = ps_s.tile([128, 258], F32, tag="s")
                    for rr in range(2):
                        r = 2 * p + rr
                        nc.tensor.matmul(
                            s_ps[:, rr * 128 : (rr + 1) * 128],
                            lhsT=ft16[:, r], rhs=gt16[:, r],
                            start=True, stop=True,
                        )
                    if p + 1 < npair:
                        hp = emit_h_pair(xt8, p + 1)
                    at2 = sb_at.tile([128, 256], BF16, tag="at2")
                    nc.scalar.activation(
                        at2[:], s_ps[:, 0:256], AF.Exp, scale=1.0 / (WS * WS)
                    )

                    for rr in range(2):
                        nc.tensor.matmul(
                            s_ps[:, 256 + rr : 257 + rr],
                            lhsT=at2[:, rr * 128 : (rr + 1) * 128],
                            rhs=onesg_sb[:],
                            start=True, stop=True,
                        )
                    scale2 = sb_small.tile([128, 2], F32, tag="scale2")
                    nc.vector.reciprocal(scale2[:], s_ps[:, 256:258])

                    for rr in range(2):
                        r = 2 * p + rr
                        o_ps = ps_o.tile([128, C], F32, tag="o")
                        nc.tensor.matmul(
                            o_ps[:], lhsT=at2[:, rr * 128 : (rr + 1) * 128],
                            rhs=h2[:, rr], start=True, stop=True,
                        )
                        nc.vector.scalar_tensor_tensor(
                            out4[:, r * C : (r + 1) * C],
                            o_ps[:],
                            scale2[:, rr : rr + 1],
                            x4[:, r * C : (r + 1) * C],
                            ALU.mult,
                            ALU.add,
                        )
                nc.sync.dma_start(out_d[rb], out4[:])
    nc.compile()
    return nc


def make_in_map(x_b: np.ndarray, wf, bf, wg, bg, wh, bh, gamma) -> dict:
    """Host-side input staging for one core (layout/dtype + constant folds)."""
    x_b = np.asarray(x_b, np.float32)
    rows = x_b.shape[0]
    RB = row_batch(rows)
    nrb = rows // RB
    gamma_f = float(np.float32(np.asarray(gamma)))
    sgn = 1.0 if gamma_f >= 0 else -1.0
    ag = max(abs(gamma_f), 1e-30)

    xt8 = np.ascontiguousarray(
        x_b.astype(E4DT)
        .reshape(nrb, RB, W, 4, 128)
        .transpose(0, 4, 3, 1, 2)
        .reshape(nrb, 128, RB * C)
    )
    x_adj = x_b + gamma_f * np.asarray(bh, np.float32)
    x4 = np.ascontiguousarray(
        x_adj.astype(BFDT)
        .reshape(nrb, RB, W, C)
        .transpose(0, 2, 1, 3)
        .reshape(nrb, 128, RB * C)
    )

    def w_dr(w_mat, scale):
        w_mat = np.asarray(w_mat, np.float32) * scale
        m = w_mat.shape[1]
        return np.ascontiguousarray(
            w_mat.astype(E4DT).reshape(4, 128, m).transpose(1, 0, 2).reshape(128, 4 * m)
        )

    return {
        "xt8": xt8,
        "x4": x4,
        "wh8": w_dr(wh, WS * sgn),
        "wf8": w_dr(wf, WS),
        "wg8": w_dr(wg, WS),
        "bf64": np.asarray(bf, np.float32).reshape(CK, 1) * WS,
        "bg64": np.asarray(bg, np.float32).reshape(CK, 1) * WS,
        "onesg": np.full((W, 1), WS / ag, np.float32).astype(BFDT),
    }


def unbatch_out(arr: np.ndarray, rows: int) -> np.ndarray:
    """[nrb, 128, RB*C] device layout -> [rows, W, C] f32."""
    RB = row_batch(rows)
    nrb = rows // RB
    return (
        np.asarray(arr)
        .astype(np.float32)
        .reshape(nrb, 128, RB, C)
        .transpose(0, 2, 1, 3)
        .reshape(rows, W, C)
    )


_NC_CACHE: dict = {}


def run(inputs: dict, trace: bool = False, **run_kwargs):
    """Build (cached), run on 8 cores, return (out, BassKernelResults)."""
    from concourse.bass_utils import run_bass_kernel_spmd

    if "nc" not in _NC_CACHE:
        _NC_CACHE["nc"] = build_nc()
    nc = _NC_CACHE["nc"]
    x = np.asarray(inputs["x"], np.float32)
    in_maps = [
        make_in_map(
            x[b],
            inputs["wf"],
            inputs["bf"],
            inputs["wg"],
            inputs["bg"],
            inputs["wh"],
            inputs["bh"],
            inputs["gamma"],
        )
        for b in range(N_CORES)
    ]
    res = run_bass_kernel_spmd(
        nc, in_maps, list(range(N_CORES)), trace=trace, **run_kwargs
    )
    out = np.stack(
        [unbatch_out(res.results[b]["out"], H) for b in range(N_CORES)], axis=0
    )
    return out, res


def kernel(**inputs) -> np.ndarray:
    out, _ = run(inputs, trace=False)
    return out


# revision 8
# speedup vs baseline: 1.3136x; 1.0486x over previous
"""Trainium2 Bass kernel for the self-attention block (nn_Attention).

Reference computation (per batch b, row h):
    f = x @ wf + bf; g = x @ wg + bg; h = x @ wh + bh      (1x1 convs)
    s = g @ f^T (over W); beta = softmax(s, -1); o = beta @ h
    out = gamma * o + x

Sharding: data-parallel over batch B=8, one batch element per NeuronCore.
Per core, each of the 128 rows is an independent [W=128, C=512] block.

v4: fp8e4 DoubleRow matmuls for the projections + a software-pipelined
pair loop tuned against measured engine rates (ACT ~263+1.07/col drain,
DVE ~190+1.18/col, stt ~750, all per the cayman SBUF-access errata).

  - x ships twice: xt8 (fp8, transposed + DoubleRow-interleaved, 8.4MB)
    feeds the PE; x4 (bf16 natural, 16.8MB) is the residual. out bf16.
  - Weights pre-scaled by 64 on the host (fp8 subnormal avoidance); the
    1/64**2 is folded into the exp scale, the 64/|gamma| into the ones
    vector of the Z-matmuls, sign(gamma) into wh, gamma*bh into x4.
  - Rows processed in PAIRS. Critical chain per pair is
    s-MM -> exp -> Z-MM -> recip -> stt; the exp is FIRST in the ACT
    queue and the h work of the NEXT pair (4 DR matmuls + its split
    ACT/DVE drain) is issued in the current step, so the PE and both
    drain engines stay busy while the chain runs.
  - h PSUM is one [128,2,C] tile (2 banks, bufs=1); it is drained in
    the step BEFORE its o-matmuls consume it, which is what lets a
    single buffer rotate without stalling the PE.
  - Z via two N=1 matmuls reusing the at2 halves as stationary (no
    second exp, no accum_out); one paired reciprocal.
"""

import numpy as np
import ml_dtypes

import concourse.bacc as bacc
import concourse.bass as bass
import concourse.mybir as mybir
import concourse.tile as tile

B, H, W, C = 8, 128, 128, 512
CK = C // 8  # 64
N_CORES = 8

F32 = mybir.dt.float32
BF16 = mybir.dt.bfloat16
FP8 = mybir.dt.float8e4
BFDT = ml_dtypes.bfloat16
E4DT = ml_dtypes.float8_e4m3
AF = mybir.ActivationFunctionType
ALU = mybir.AluOpType
DR = mybir.MatmulPerfMode.DoubleRow

WS = 64.0    # host-side weight scale
H_ACT = 320  # columns (of 512) of each h half-drain handled by ScalarE


def row_batch(rows: int) -> int:
    for rb in (4, 2):
        if rows % rb == 0:
            return rb
    return 1


def build_nc(rows: int = H) -> bass.Bass:
    nc = bacc.Bacc(None)
    RB = row_batch(rows)
    nrb = rows // RB
    npair = RB // 2
    assert npair, "rows must be a multiple of 2"
    xt8_d = nc.dram_tensor("xt8", [nrb, 128, RB * C], FP8, kind="ExternalInput")
    x4_d = nc.dram_tensor("x4", [nrb, 128, RB * C], BF16, kind="ExternalInput")
    wh8_d = nc.dram_tensor("wh8", [128, 2 * 2 * C], FP8, kind="ExternalInput")
    wf8_d = nc.dram_tensor("wf8", [128, 2 * 2 * CK], FP8, kind="ExternalInput")
    wg8_d = nc.dram_tensor("wg8", [128, 2 * 2 * CK], FP8, kind="ExternalInput")
    bf64_d = nc.dram_tensor("bf64", [CK, 1], F32, kind="ExternalInput")
    bg64_d = nc.dram_tensor("bg64", [CK, 1], F32, kind="ExternalInput")
    onesg_d = nc.dram_tensor("onesg", [W, 1], BF16, kind="ExternalInput")
    out_d = nc.dram_tensor("out", [nrb, 128, RB * C], BF16, kind="ExternalOutput")

    with tile.TileContext(nc) as tc:
        with (
            tc.tile_pool(name="const", bufs=1) as cpool,
            tc.tile_pool(name="sb_xt", bufs=3) as sb_xt,
            tc.tile_pool(name="sb_x", bufs=3) as sb_x,
            tc.tile_pool(name="sb_fg", bufs=2) as sb_fg,
            tc.tile_pool(name="sb_h", bufs=3) as sb_h,
            tc.tile_pool(name="sb_at", bufs=3) as sb_at,
            tc.tile_pool(name="sb_out", bufs=2) as sb_out,
            tc.tile_pool(name="sb_small", bufs=6) as sb_small,
            tc.tile_pool(name="ps_f", bufs=1, space="PSUM") as ps_f,
            tc.tile_pool(name="ps_g", bufs=1, space="PSUM") as ps_g,
            tc.tile_pool(name="ps_h", bufs=1, space="PSUM") as ps_h,
            tc.tile_pool(name="ps_s", bufs=2, space="PSUM") as ps_s,
            tc.tile_pool(name="ps_o", bufs=2, space="PSUM") as ps_o,
        ):
            wh8_sb = cpool.tile([128, 2, 2, C], FP8)
            nc.sync.dma_start(wh8_sb[:], wh8_d[:])
            wf8_sb = cpool.tile([128, 2, 2, CK], FP8)
            nc.sync.dma_start(wf8_sb[:], wf8_d[:])
            wg8_sb = cpool.tile([128, 2, 2, CK], FP8)
            nc.sync.dma_start(wg8_sb[:], wg8_d[:])
            bf64_sb = cpool.tile([CK, 1], F32)
            nc.sync.dma_start(bf64_sb[:], bf64_d[:])
            bg64_sb = cpool.tile([CK, 1], F32)
            nc.sync.dma_start(bg64_sb[:], bg64_d[:])
            onesg_sb = cpool.tile([W, 1], BF16)
            nc.sync.dma_start(onesg_sb[:], onesg_d[:])

            def start_rb(rb):
                """DMAs + f/g projections for one 4-row batch."""
                st = {}
                st["xt8"] = sb_xt.tile(
                    [128, 2, 2, RB, 128], FP8, tag="xt8", name="xt8_t"
                )
                nc.gpsimd.dma_start(st["xt8"][:], xt8_d[rb])
                st["x4"] = sb_x.tile([128, RB * C], BF16, tag="x4", name="x4_t")
                nc.scalar.dma_start(st["x4"][:], x4_d[rb])
                st["out4"] = sb_out.tile(
                    [128, RB * C], BF16, tag="out4", name="out4_t"
                )
                st["rb"] = rb
                fA = ps_f.tile([CK, RB * 128], F32, tag="fA", name="fA_t")
                gA = ps_g.tile([CK, RB * 128], F32, tag="gA", name="gA_t")
                for j in range(2):
                    nc.tensor.matmul(
                        fA[:], lhsT=wf8_sb[:, j], rhs=st["xt8"][:, j],
                        start=(j == 0), stop=(j == 1), perf_mode=DR,
                    )
                for j in range(2):
                    nc.tensor.matmul(
                        gA[:], lhsT=wg8_sb[:, j], rhs=st["xt8"][:, j],
                        start=(j == 0), stop=(j == 1), perf_mode=DR,
                    )
                st["ft16"] = sb_fg.tile([CK, RB, 128], BF16, tag="ft16", name="ft_t")
                nc.scalar.activation(
                    st["ft16"][:], fA[:], AF.Identity, bias=bf64_sb[:]
                )
                st["gt16"] = sb_fg.tile([CK, RB, 128], BF16, tag="gt16", name="gt_t")
                nc.scalar.activation(
                    st["gt16"][:], gA[:], AF.Identity, bias=bg64_sb[:]
                )
                return st

            def emit_h(st, p):
                """h matmuls for pair p of batch st, plus the split drain."""
                hp = ps_h.tile([128, 2, C], F32, tag="h", name="h_ps")
                for rr in range(2):
                    for j in range(2):
                        nc.tensor.matmul(
                            hp[:, rr], lhsT=st["xt8"][:, j, :, 2 * p + rr, :],
                            rhs=wh8_sb[:, j],
                            start=(j == 0), stop=(j == 1), perf_mode=DR,
                        )
                h2 = sb_h.tile([128, 2, C], BF16, tag="h2", name="h2_t")
                nc.vector.tensor_copy(h2[:, :, H_ACT:C], hp[:, :, H_ACT:C])
                nc.scalar.activation(
                    h2[:, :, 0:H_ACT], hp[:, :, 0:H_ACT], AF.Identity
                )
                return h2

            pairs = [(rb, p) for rb in range(nrb) for p in range(npair)]
            cur = start_rb(0)
            h2_cur = emit_h(cur, 0)
            for rb, p in pairs:
                # s^T for both rows of the pair -> one PSUM tile (+2 z cols)
                s_ps = ps_s.tile([128, 258], F32, tag="s", name="s_ps")
                for rr in range(2):
                    r = 2 * p + rr
                    nc.tensor.matmul(
                        s_ps[:, rr * 128 : (rr + 1) * 128],
                        lhsT=cur["ft16"][:, r], rhs=cur["gt16"][:, r],
                        start=True, stop=True,
                    )
                at2 = sb_at.tile([128, 256], BF16, tag="at2", name="at2_t")
                nc.scalar.activation(
                    at2[:], s_ps[:, 0:256], AF.Exp, scale=1.0 / (WS * WS)
                )

                # issue next pair's h (matmuls + drain) to cover the chain
                nxt = None
                h2_next = None
                if p + 1 < npair:
                    h2_next = emit_h(cur, p + 1)
                elif rb + 1 < nrb:
                    nxt = start_rb(rb + 1)
                    h2_next = emit_h(nxt, 0)

                for rr in range(2):
                    nc.tensor.matmul(
                        s_ps[:, 256 + rr : 257 + rr],
                        lhsT=at2[:, rr * 128 : (rr + 1) * 128],
                        rhs=onesg_sb[:],
                        start=True, stop=True,
                    )
                scale2 = sb_small.tile([128, 2], F32, tag="scale2", name="sc_t")
                nc.vector.reciprocal(scale2[:], s_ps[:, 256:258])

                for rr in range(2):
                    r = 2 * p + rr
                    o_ps = ps_o.tile([128, C], F32, tag="o", name="o_ps")
                    nc.tensor.matmul(
                        o_ps[:], lhsT=at2[:, rr * 128 : (rr + 1) * 128],
                        rhs=h2_cur[:, rr], start=True, stop=True,
                    )
                    nc.vector.scalar_tensor_tensor(
                        cur["out4"][:, r * C : (r + 1) * C],
                        o_ps[:],
                        scale2[:, rr : rr + 1],
                        cur["x4"][:, r * C : (r + 1) * C],
                        ALU.mult,
                        ALU.add,
                    )
                if p == npair - 1:
                    nc.sync.dma_start(out_d[cur["rb"]], cur["out4"][:])
                    if nxt is not None:
                        cur = nxt
                h2_cur = h2_next
    nc.compile()
    return nc


def make_in_map(x_b: np.ndarray, wf, bf, wg, bg, wh, bh, gamma) -> dict:
    """Host-side input staging for one core (layout/dtype + constant folds)."""
    x_b = np.asarray(x_b, np.float32)
    rows = x_b.shape[0]
    RB = row_batch(rows)
    nrb = rows // RB
    gamma_f = float(np.float32(np.asarray(gamma)))
    sgn = 1.0 if gamma_f >= 0 else -1.0
    ag = max(abs(gamma_f), 1e-30)

    xt8 = np.ascontiguousarray(
        x_b.astype(E4DT)
        .reshape(nrb, RB, W, 4, 128)
        .transpose(0, 4, 3, 1, 2)
        .reshape(nrb, 128, RB * C)
    )
    x_adj = x_b + gamma_f * np.asarray(bh, np.float32)
    x4 = np.ascontiguousarray(
        x_adj.astype(BFDT)
        .reshape(nrb, RB, W, C)
        .transpose(0, 2, 1, 3)
        .reshape(nrb, 128, RB * C)
    )

    def w_dr(w_mat, scale):
        w_mat = np.asarray(w_mat, np.float32) * scale
        m = w_mat.shape[1]
        return np.ascontiguousarray(
            w_mat.astype(E4DT).reshape(4, 128, m).transpose(1, 0, 2).reshape(128, 4 * m)
        )

    return {
        "xt8": xt8,
        "x4": x4,
        "wh8": w_dr(wh, WS * sgn),
        "wf8": w_dr(wf, WS),
        "wg8": w_dr(wg, WS),
        "bf64": np.asarray(bf, np.float32).reshape(CK, 1) * WS,
        "bg64": np.asarray(bg, np.float32).reshape(CK, 1) * WS,
        "onesg": np.full((W, 1), WS / ag, np.float32).astype(BFDT),
    }


def unbatch_out(arr: np.ndarray, rows: int) -> np.ndarray:
    """[nrb, 128, RB*C] device layout -> [rows, W, C] f32."""
    RB = row_batch(rows)
    nrb = rows // RB
    return (
        np.asarray(arr)
        .astype(np.float32)
        .reshape(nrb, 128, RB, C)
        .transpose(0, 2, 1, 3)
        .reshape(rows, W, C)
    )


_NC_CACHE: dict = {}


def run(inputs: dict, trace: bool = False, **run_kwargs):
    """Build (cached), run on 8 cores, return (out, BassKernelResults)."""
    from concourse.bass_utils import run_bass_kernel_spmd

    if "nc" not in _NC_CACHE:
        _NC_CACHE["nc"] = build_nc()
    nc = _NC_CACHE["nc"]
    x = np.asarray(inputs["x"], np.float32)
    in_maps = [
        make_in_map(
            x[b],
            inputs["wf"],
            inputs["bf"],
            inputs["wg"],
            inputs["bg"],
            inputs["wh"],
            inputs["bh"],
            inputs["gamma"],
        )
        for b in range(N_CORES)
    ]
    res = run_bass_kernel_spmd(
        nc, in_maps, list(range(N_CORES)), trace=trace, **run_kwargs
    )
    out = np.stack(
        [unbatch_out(res.results[b]["out"], H) for b in range(N_CORES)], axis=0
    )
    return out, res


def kernel(**inputs) -> np.ndarray:
    out, _ = run(inputs, trace=False)
    return out


# revision 9
# speedup vs baseline: 1.3481x; 1.0263x over previous
"""Trainium2 Bass kernel for the self-attention block (nn_Attention).

Reference computation (per batch b, row h):
    f = x @ wf + bf; g = x @ wg + bg; h = x @ wh + bh      (1x1 convs)
    s = g @ f^T (over W); beta = softmax(s, -1); o = beta @ h
    out = gamma * o + x

Sharding: data-parallel over batch B=8, one batch element per NeuronCore.
Per core, each of the 128 rows is an independent [W=128, C=512] block.

v4: fp8e4 DoubleRow matmuls for the projections + a software-pipelined
pair loop tuned against measured engine rates (ACT ~263+1.07/col drain,
DVE ~190+1.18/col, stt ~750, all per the cayman SBUF-access errata).

  - x ships twice: xt8 (fp8, transposed + DoubleRow-interleaved, 8.4MB)
    feeds the PE; x4 (bf16 natural, 16.8MB) is the residual. out bf16.
  - Weights pre-scaled by 64 on the host (fp8 subnormal avoidance); the
    1/64**2 is folded into the exp scale, the 64/|gamma| into the ones
    vector of the Z-matmuls, sign(gamma) into wh, gamma*bh into x4.
  - Rows processed in PAIRS. Critical chain per pair is
    s-MM -> exp -> Z-MM -> recip -> stt; the exp is FIRST in the ACT
    queue and the h work of the NEXT pair (4 DR matmuls + its split
    ACT/DVE drain) is issued in the current step, so the PE and both
    drain engines stay busy while the chain runs.
  - h PSUM is one [128,2,C] tile (2 banks, bufs=1); it is drained in
    the step BEFORE its o-matmuls consume it, which is what lets a
    single buffer rotate without stalling the PE.
  - Z via two N=1 matmuls reusing the at2 halves as stationary (no
    second exp, no accum_out); one paired reciprocal.
"""

import numpy as np
import ml_dtypes

import concourse.bacc as bacc
import concourse.bass as bass
import concourse.mybir as mybir
import concourse.tile as tile

B, H, W, C = 8, 128, 128, 512
CK = C // 8  # 64
N_CORES = 8

F32 = mybir.dt.float32
BF16 = mybir.dt.bfloat16
FP8 = mybir.dt.float8e4
BFDT = ml_dtypes.bfloat16
E4DT = ml_dtypes.float8_e4m3
AF = mybir.ActivationFunctionType
ALU = mybir.AluOpType
DR = mybir.MatmulPerfMode.DoubleRow

WS = 64.0    # host-side weight scale
H_ACT = 384  # columns (of 512) of each h half-drain handled by ScalarE


def row_batch(rows: int) -> int:
    for rb in (4, 2):
        if rows % rb == 0:
            return rb
    return 1


def build_nc(rows: int = H) -> bass.Bass:
    nc = bacc.Bacc(None)
    RB = row_batch(rows)
    nrb = rows // RB
    npair = RB // 2
    assert npair, "rows must be a multiple of 2"
    xt8_d = nc.dram_tensor("xt8", [nrb, 128, RB * C], FP8, kind="ExternalInput")
    x4_d = nc.dram_tensor("x4", [nrb, 128, RB * C], BF16, kind="ExternalInput")
    wh8_d = nc.dram_tensor("wh8", [128, 2 * 2 * C], FP8, kind="ExternalInput")
    wf8_d = nc.dram_tensor("wf8", [128, 2 * 2 * CK], FP8, kind="ExternalInput")
    wg8_d = nc.dram_tensor("wg8", [128, 2 * 2 * CK], FP8, kind="ExternalInput")
    bf64_d = nc.dram_tensor("bf64", [CK, 1], F32, kind="ExternalInput")
    bg64_d = nc.dram_tensor("bg64", [CK, 1], F32, kind="ExternalInput")
    onesg_d = nc.dram_tensor("onesg", [W, 1], BF16, kind="ExternalInput")
    out_d = nc.dram_tensor("out", [nrb, 128, RB * C], BF16, kind="ExternalOutput")

    with tile.TileContext(nc) as tc:
        with (
            tc.tile_pool(name="const", bufs=1) as cpool,
            tc.tile_pool(name="sb_xt", bufs=3) as sb_xt,
            tc.tile_pool(name="sb_x", bufs=3) as sb_x,
            tc.tile_pool(name="sb_fg", bufs=2) as sb_fg,
            tc.tile_pool(name="sb_h", bufs=3) as sb_h,
            tc.tile_pool(name="sb_at", bufs=3) as sb_at,
            tc.tile_pool(name="sb_out", bufs=2) as sb_out,
            tc.tile_pool(name="sb_small", bufs=6) as sb_small,
            tc.tile_pool(name="ps_f", bufs=1, space="PSUM") as ps_f,
            tc.tile_pool(name="ps_g", bufs=1, space="PSUM") as ps_g,
            tc.tile_pool(name="ps_h", bufs=1, space="PSUM") as ps_h,
            tc.tile_pool(name="ps_s", bufs=2, space="PSUM") as ps_s,
            tc.tile_pool(name="ps_o", bufs=2, space="PSUM") as ps_o,
        ):
            wh8_sb = cpool.tile([128, 2, 2, C], FP8)
            nc.sync.dma_start(wh8_sb[:], wh8_d[:])
            wf8_sb = cpool.tile([128, 2, 2, CK], FP8)
            nc.sync.dma_start(wf8_sb[:], wf8_d[:])
            wg8_sb = cpool.tile([128, 2, 2, CK], FP8)
            nc.sync.dma_start(wg8_sb[:], wg8_d[:])
            bf64_sb = cpool.tile([CK, 1], F32)
            nc.sync.dma_start(bf64_sb[:], bf64_d[:])
            bg64_sb = cpool.tile([CK, 1], F32)
            nc.sync.dma_start(bg64_sb[:], bg64_d[:])
            onesg_sb = cpool.tile([W, 1], BF16)
            nc.sync.dma_start(onesg_sb[:], onesg_d[:])

            def start_rb(rb):
                """DMAs + f/g projections for one 4-row batch."""
                st = {}
                st["xt8"] = sb_xt.tile(
                    [128, 2, 2, RB, 128], FP8, tag="xt8", name="xt8_t"
                )
                nc.gpsimd.dma_start(st["xt8"][:], xt8_d[rb])
                st["x4"] = sb_x.tile([128, RB * C], BF16, tag="x4", name="x4_t")
                nc.scalar.dma_start(st["x4"][:], x4_d[rb])
                st["out4"] = sb_out.tile(
                    [128, RB * C], BF16, tag="out4", name="out4_t"
                )
                st["rb"] = rb
                fA = ps_f.tile([CK, RB * 128], F32, tag="fA", name="fA_t")
                gA = ps_g.tile([CK, RB * 128], F32, tag="gA", name="gA_t")
                for j in range(2):
                    nc.tensor.matmul(
                        fA[:], lhsT=wf8_sb[:, j], rhs=st["xt8"][:, j],
                        start=(j == 0), stop=(j == 1), perf_mode=DR,
                    )
                for j in range(2):
                    nc.tensor.matmul(
                        gA[:], lhsT=wg8_sb[:, j], rhs=st["xt8"][:, j],
                        start=(j == 0), stop=(j == 1), perf_mode=DR,
                    )
                st["ft16"] = sb_fg.tile([CK, RB, 128], BF16, tag="ft16", name="ft_t")
                nc.scalar.activation(
                    st["ft16"][:], fA[:], AF.Identity, bias=bf64_sb[:]
                )
                st["gt16"] = sb_fg.tile([CK, RB, 128], BF16, tag="gt16", name="gt_t")
                nc.scalar.activation(
                    st["gt16"][:], gA[:], AF.Identity, bias=bg64_sb[:]
                )
                return st

            def emit_h(st, p):
                """h matmuls for pair p of batch st, plus the split drain."""
                hp = ps_h.tile([128, 2, C], F32, tag="h", name="h_ps")
                for rr in range(2):
                    for j in range(2):
                        nc.tensor.matmul(
                            hp[:, rr], lhsT=st["xt8"][:, j, :, 2 * p + rr, :],
                            rhs=wh8_sb[:, j],
                            start=(j == 0), stop=(j == 1), perf_mode=DR,
                        )
                h2 = sb_h.tile([128, 2, C], BF16, tag="h2", name="h2_t")
                nc.vector.tensor_copy(h2[:, :, H_ACT:C], hp[:, :, H_ACT:C])
                nc.scalar.activation(
                    h2[:, :, 0:H_ACT], hp[:, :, 0:H_ACT], AF.Identity
                )
                return h2

            pairs = [(rb, p) for rb in range(nrb) for p in range(npair)]

            def stage_b(e):
                """Z + recip + o + epilogue for a pair whose exp already ran."""
                st, p, s_ps, at2, h2 = e["st"], e["p"], e["s_ps"], e["at2"], e["h2"]
                for rr in range(2):
                    nc.tensor.matmul(
                        s_ps[:, 256 + rr : 257 + rr],
                        lhsT=at2[:, rr * 128 : (rr + 1) * 128],
                        rhs=onesg_sb[:],
                        start=True, stop=True,
                    )
                scale2 = sb_small.tile([128, 2], F32, tag="scale2", name="sc_t")
                nc.vector.reciprocal(scale2[:], s_ps[:, 256:258])
                for rr in range(2):
                    r = 2 * p + rr
                    o_ps = ps_o.tile([128, C], F32, tag="o", name="o_ps")
                    nc.tensor.matmul(
                        o_ps[:], lhsT=at2[:, rr * 128 : (rr + 1) * 128],
                        rhs=h2[:, rr], start=True, stop=True,
                    )
                    nc.vector.scalar_tensor_tensor(
                        st["out4"][:, r * C : (r + 1) * C],
                        o_ps[:],
                        scale2[:, rr : rr + 1],
                        st["x4"][:, r * C : (r + 1) * C],
                        ALU.mult,
                        ALU.add,
                    )
                if p == npair - 1:
                    nc.sync.dma_start(out_d[st["rb"]], st["out4"][:])

            cur = start_rb(0)
            h2_next = emit_h(cur, 0)
            prev = None
            for rb, p in pairs:
                st = cur
                h2_this = h2_next
                s_ps = ps_s.tile([128, 258], F32, tag="s", name="s_ps")
                for rr in range(2):
                    r = 2 * p + rr
                    nc.tensor.matmul(
                        s_ps[:, rr * 128 : (rr + 1) * 128],
                        lhsT=st["ft16"][:, r], rhs=st["gt16"][:, r],
                        start=True, stop=True,
                    )
                at2 = sb_at.tile([128, 256], BF16, tag="at2", name="at2_t")
                nc.scalar.activation(
                    at2[:], s_ps[:, 0:256], AF.Exp, scale=1.0 / (WS * WS)
                )
                # prefetch next pair's h (matmuls + drain) and next rb state
                if p + 1 < npair:
                    h2_next = emit_h(st, p + 1)
                elif rb + 1 < nrb:
                    cur = start_rb(rb + 1)
                    h2_next = emit_h(cur, 0)
                else:
                    h2_next = None
                if prev is not None:
                    stage_b(prev)
                prev = {"st": st, "p": p, "s_ps": s_ps, "at2": at2, "h2": h2_this}
            stage_b(prev)
    nc.compile()
    return nc


def make_in_map(x_b: np.ndarray, wf, bf, wg, bg, wh, bh, gamma) -> dict:
    """Host-side input staging for one core (layout/dtype + constant folds)."""
    x_b = np.asarray(x_b, np.float32)
    rows = x_b.shape[0]
    RB = row_batch(rows)
    nrb = rows // RB
    gamma_f = float(np.float32(np.asarray(gamma)))
    sgn = 1.0 if gamma_f >= 0 else -1.0
    ag = max(abs(gamma_f), 1e-30)

    xt8 = np.ascontiguousarray(
        x_b.astype(E4DT)
        .reshape(nrb, RB, W, 4, 128)
        .transpose(0, 4, 3, 1, 2)
        .reshape(nrb, 128, RB * C)
    )
    x_adj = x_b + gamma_f * np.asarray(bh, np.float32)
    x4 = np.ascontiguousarray(
        x_adj.astype(BFDT)
        .reshape(nrb, RB, W, C)
        .transpose(0, 2, 1, 3)
        .reshape(nrb, 128, RB * C)
    )

    def w_dr(w_mat, scale):
        w_mat = np.asarray(w_mat, np.float32) * scale
        m = w_mat.shape[1]
        return np.ascontiguousarray(
            w_mat.astype(E4DT).reshape(4, 128, m).transpose(1, 0, 2).reshape(128, 4 * m)
        )

    return {
        "xt8": xt8,
        "x4": x4,
        "wh8": w_dr(wh, WS * sgn),
        "wf8": w_dr(wf, WS),
        "wg8": w_dr(wg, WS),
        "bf64": np.asarray(bf, np.float32).reshape(CK, 1) * WS,
        "bg64": np.asarray(bg, np.float32).reshape(CK, 1) * WS,
        "onesg": np.full((W, 1), WS / ag, np.float32).astype(BFDT),
    }


def unbatch_out(arr: np.ndarray, rows: int) -> np.ndarray:
    """[nrb, 128, RB*C] device layout -> [rows, W, C] f32."""
    RB = row_batch(rows)
    nrb = rows // RB
    return (
        np.asarray(arr)
        .astype(np.float32)
        .reshape(nrb, 128, RB, C)
        .transpose(0, 2, 1, 3)
        .reshape(rows, W, C)
    )


_NC_CACHE: dict = {}


def run(inputs: dict, trace: bool = False, **run_kwargs):
    """Build (cached), run on 8 cores, return (out, BassKernelResults)."""
    from concourse.bass_utils import run_bass_kernel_spmd

    if "nc" not in _NC_CACHE:
        _NC_CACHE["nc"] = build_nc()
    nc = _NC_CACHE["nc"]
    x = np.asarray(inputs["x"], np.float32)
    in_maps = [
        make_in_map(
            x[b],
            inputs["wf"],
            inputs["bf"],
            inputs["wg"],
            inputs["bg"],
            inputs["wh"],
            inputs["bh"],
            inputs["gamma"],
        )
        for b in range(N_CORES)
    ]
    res = run_bass_kernel_spmd(
        nc, in_maps, list(range(N_CORES)), trace=trace, **run_kwargs
    )
    out = np.stack(
        [unbatch_out(res.results[b]["out"], H) for b in range(N_CORES)], axis=0
    )
    return out, res


def kernel(**inputs) -> np.ndarray:
    out, _ = run(inputs, trace=False)
    return out
